# revision 4
# baseline (speedup 1.0000x reference)
"""Deep Markov Model ELBO kernel for 8 Trainium2 NeuronCores.

Strategy: data-parallel over batch (512 -> 64 rows/core). Per core:
  Phase A (sequential chain, T steps): LSTM recurrence + combiner z-chain,
    fp32r matmuls with activations as the stationary operand (M=64) and
    weights as the moving operand (N<=512). Sigmoid expressed via tanh so a
    single ACT table set (exp_and_others) serves the whole kernel. Biases
    enter PSUM as K=1 matmuls. Recurrent state kept transposed (feat x batch)
    via PE transposes; the 1/2 factors of the tanh-sigmoid algebra are folded
    into host-preprocessed Whh / Wc0h.
  Phase B (batched): transition + emission MLPs over all T*64 z-vectors at
    M=128 efficiency; diagonal-Gaussian log-prob sums reduced per column with
    ones-vector matmuls.
Host (numpy, f64): eps^2 row sums, the prior log-prob at t=0, kl = sum of
max(lq - lp, free_bits), and the final mean over the batch.
"""
import os
import sys

import numpy as np

for _p in ("/opt/trn_rl_repo", "/root/.axon_site/_ro/trn_rl_repo"):
    if os.path.isdir(_p) and _p not in sys.path:
        sys.path.insert(0, _p)

import concourse.bacc as bacc
import concourse.mybir as mybir
from concourse import tile
from concourse.bass_utils import run_bass_kernel_spmd

F32 = mybir.dt.float32
F32R = mybir.dt.float32r
BF16 = mybir.dt.bfloat16
AF = mybir.ActivationFunctionType
ALU = mybir.AluOpType

B, D, L, H = 512, 128, 128, 512
NCORES = 8
BS = B // NCORES  # 64 batch rows per core
LOG2PI = float(np.log(2.0 * np.pi))
FREE_BITS = 0.1 * L

_DT_ENV = os.environ.get("DMM_DT", "f32r")
DT = {"f32r": F32R, "f32": F32, "bf16": BF16}[_DT_ENV]  # matmul operand dtype
import ml_dtypes as _mld
NPDT = {"f32r": np.float32, "f32": np.float32,
        "bf16": _mld.bfloat16}[_DT_ENV]   # host dtype for DT tensors

_CACHE = {}

CHAIN_W = ["wih", "whh", "blstm", "wc0z", "wc0h", "bc0", "wc1", "bc1", "wc2",
           "bc2", "ones_row", "ident"]
BATCH_W = ["wt0", "wt1", "wt2", "we0", "we1", "we2", "ones_col", "bt0c",
           "bt1c", "btlocc", "btlsnc", "btlsc", "be0c", "be1c", "belsnc",
           "belsc"]


def _build(T, run_chain=True):
    """Emit the bass program for a T-step chain. Same program on all cores."""
    nc = bacc.Bacc(None, target_bir_lowering=False, debug=False)
    NCH = (T * BS) // 512  # batched-pass chunks
    assert (T * BS) % 512 == 0

    din = {}

    def inp(name, shape, dt=DT):
        din[name] = nc.dram_tensor(name, list(shape), dt, kind="ExternalInput")
        return din[name]

    # weights (transposed on host; Whh/Wc0h pre-halved; gates reordered i,f,o,g)
    inp("wih", (128, 4 * H))
    inp("whh", (128, 4, 4 * H))
    inp("blstm", (1, 4 * H))
    inp("wc0z", (128, H))
    inp("wc0h", (128, 4, H))
    inp("bc0", (1, H))
    inp("wc1", (128, 4, H))
    inp("bc1", (1, H))
    inp("wc2", (128, 4, 2 * L))
    inp("bc2", (1, 2 * L))
    inp("wt0", (128, H))
    inp("wt1", (128, 4, H))
    inp("wt2", (128, 4, 2 * L))
    inp("we0", (128, H))
    inp("we1", (128, 4, H))
    inp("we2", (128, 4, 2 * L))
    inp("ones_row", (1, BS))        # bias-MM stationary (chain)
    inp("ones_col", (128, 1))       # reduce-MM stationary (batched)
    inp("ident", (128, 128), F32)   # PE transpose identity
    inp("bt0c", (128, 4), F32)
    inp("bt1c", (128, 4), F32)
    inp("btlocc", (128, 1), F32)
    inp("btlsnc", (128, 1), F32)    # -b_ls (transition)
    inp("btlsc", (128, 1), F32)     # +b_ls
    inp("be0c", (128, 4), F32)
    inp("be1c", (128, 4), F32)
    inp("belsnc", (128, 1), F32)
    inp("belsc", (128, 1), F32)
    # per-core data
    xT = inp("xT", (128, T, BS))               # x shard, (d, t, b)
    xadj = inp("xadj", (128, BS * T), F32)     # (d, b-major cols), minus emis loc bias
    epsn = inp("epsn", (T, BS, L), F32)        # eps shard, natural layout

    sqls_out = nc.dram_tensor("sqls_out", [BS, T], F32, kind="ExternalOutput")
    lp_out = nc.dram_tensor("lp_out", [1, BS * T], F32, kind="ExternalOutput")
    recon_out = nc.dram_tensor("recon_out", [1, 512], F32, kind="ExternalOutput")
    z1t_out = nc.dram_tensor("z1t_out", [128, BS], F32, kind="ExternalOutput")

    with tile.TileContext(nc) as tc:
        with tc.tile_pool(name="zsb", bufs=1) as zsb:
            zbuf = zsb.tile([128, BS * T + 1], DT, tag="zbuf")  # col0 = pad
            zview = zbuf[:, 1:BS * T + 1].rearrange("p (b t) -> p b t", t=T)
            sqls = zsb.tile([BS, T], F32, tag="sqls")
            nc.vector.memset(zbuf[:, 0:1].bitcast(F32), 0.0)

            # ---------------- Phase A: the chain ----------------
            with (
                tc.tile_pool(name="wA", bufs=1) as wA,
                tc.tile_pool(name="gps", bufs=4, space="PSUM") as gps_pool,
                tc.tile_pool(name="mps", bufs=2, space="PSUM") as mps_pool,
                tc.tile_pool(name="tps", bufs=2, space="PSUM") as tps_pool,
                tc.tile_pool(name="io", bufs=3) as io_pool,
                tc.tile_pool(name="gact", bufs=2) as gact_pool,
                tc.tile_pool(name="tmp", bufs=2) as tmp_pool,
                tc.tile_pool(name="st", bufs=2) as st_pool,
            ):
                w = {}
                for name in CHAIN_W:
                    dten = din[name]
                    tl = wA.tile(list(dten.shape), dten.dtype, tag=f"w_{name}")
                    nc.sync.dma_start(tl[:], dten[:])
                    w[name] = tl
                ident = w["ident"]

                hT_prev = None
                cC_prev = None
                zT_prev = None
                for t in range(T):
                    xt = io_pool.tile([128, BS], DT, tag="xt")
                    nc.sync.dma_start(xt[:], xT[:, t, :])
                    ep = io_pool.tile([BS, L], F32, tag="ep")
                    nc.sync.dma_start(ep[:], epsn[t])

                    # -- LSTM gates: PSUM = x@Wih.T + h@Whh.T + b (512 chunks)
                    g_ps = []
                    for ch in range(4):
                        p = gps_pool.tile([BS, 512], F32, tag="gp")
                        cs = slice(ch * 512, (ch + 1) * 512)
                        nc.tensor.matmul(p[:], xt[:], w["wih"][:, cs],
                                         start=True, stop=False)
                        if t > 0:
                            for k in range(4):
                                nc.tensor.matmul(
                                    p[:], hT_prev[:, k * BS:(k + 1) * BS],
                                    w["whh"][:, k, cs], start=False, stop=False)
                        nc.tensor.matmul(p[:], w["ones_row"][:],
                                         w["blstm"][:, cs], start=False, stop=True)
                        g_ps.append(p)

                    # -- activations: chunks are [i, f, o, g] (host-reordered)
                    ti = gact_pool.tile([BS, 512], F32, tag="ti")
                    tf = gact_pool.tile([BS, 512], F32, tag="tf")
                    to = gact_pool.tile([BS, 512], F32, tag="to")
                    tg = gact_pool.tile([BS, 512], F32, tag="tg")
                    nc.scalar.activation(ti[:], g_ps[0][:], AF.Tanh, scale=0.5)
                    nc.scalar.activation(tf[:], g_ps[1][:], AF.Tanh, scale=0.5)
                    nc.scalar.activation(to[:], g_ps[2][:], AF.Tanh, scale=0.5)
                    nc.scalar.activation(tg[:], g_ps[3][:], AF.Tanh)

                    # -- cell update (carry cC = 2c): cC' = 0.5(1+tf)cC + (1+ti)tg
                    v2 = tmp_pool.tile([BS, 512], F32, tag="v2")
                    nc.vector.scalar_tensor_tensor(
                        v2[:], ti[:], 1.0, tg[:], op0=ALU.add, op1=ALU.mult)
                    cC = st_pool.tile([BS, 512], F32, tag="cC")
                    if t == 0:
                        nc.vector.tensor_copy(cC[:], v2[:])
                    else:
                        v1 = tmp_pool.tile([BS, 512], F32, tag="v1")
                        nc.vector.scalar_tensor_tensor(
                            v1[:], tf[:], 1.0, cC_prev[:], op0=ALU.add, op1=ALU.mult)
                        nc.vector.scalar_tensor_tensor(
                            cC[:], v1[:], 0.5, v2[:], op0=ALU.mult, op1=ALU.add)
                    tc_t = tmp_pool.tile([BS, 512], F32, tag="tc_t")
                    nc.scalar.activation(tc_t[:], cC[:], AF.Tanh, scale=0.5)
                    # hH = 2h = (1+to)*tanh(c); the 1/2 is folded into Whh/Wc0h
                    hH = tmp_pool.tile([BS, 512], F32, tag="hH")
                    nc.vector.scalar_tensor_tensor(
                        hH[:], to[:], 1.0, tc_t[:], op0=ALU.add, op1=ALU.mult)

                    # -- transpose h to (feat, batch) for next stationaries
                    hT_ps = tps_pool.tile([128, 4 * BS], F32, tag="tp")
                    for k in range(4):
                        nc.tensor.transpose(
                            hT_ps[:, k * BS:(k + 1) * BS],
                            hH[:, k * 128:(k + 1) * 128], ident[:BS, :BS])
                    hT = st_pool.tile([128, 4 * BS], DT, tag="hT")
                    nc.vector.tensor_copy(hT[:], hT_ps[:])

                    # -- combiner layer 0
                    l0 = mps_pool.tile([BS, 512], F32, tag="mp")
                    if t > 0:
                        nc.tensor.matmul(l0[:], zT_prev[:], w["wc0z"][:],
                                         start=True, stop=False)
                    for k in range(4):
                        nc.tensor.matmul(l0[:], hT[:, k * BS:(k + 1) * BS],
                                         w["wc0h"][:, k, :],
                                         start=(t == 0 and k == 0), stop=False)
                    nc.tensor.matmul(l0[:], w["ones_row"][:], w["bc0"][:],
                                     start=False, stop=True)
                    a0 = tmp_pool.tile([BS, 512], F32, tag="a0")
                    nc.scalar.activation(a0[:], l0[:], AF.Relu)
                    a0T_ps = tps_pool.tile([128, 4 * BS], F32, tag="tp")
                    for k in range(4):
                        nc.tensor.transpose(
                            a0T_ps[:, k * BS:(k + 1) * BS],
                            a0[:, k * 128:(k + 1) * 128], ident[:BS, :BS])
                    a0T = st_pool.tile([128, 4 * BS], DT, tag="a0T")
                    nc.vector.tensor_copy(a0T[:], a0T_ps[:])

                    # -- combiner layer 1
                    l1 = mps_pool.tile([BS, 512], F32, tag="mp")
                    for k in range(4):
                        nc.tensor.matmul(l1[:], a0T[:, k * BS:(k + 1) * BS],
                                         w["wc1"][:, k, :],
                                         start=(k == 0), stop=False)
                    nc.tensor.matmul(l1[:], w["ones_row"][:], w["bc1"][:],
                                     start=False, stop=True)
                    a1 = tmp_pool.tile([BS, 512], F32, tag="a1")
                    nc.scalar.activation(a1[:], l1[:], AF.Relu)
                    a1T_ps = tps_pool.tile([128, 4 * BS], F32, tag="tp")
                    for k in range(4):
                        nc.tensor.transpose(
                            a1T_ps[:, k * BS:(k + 1) * BS],
                            a1[:, k * 128:(k + 1) * 128], ident[:BS, :BS])
                    a1T = st_pool.tile([128, 4 * BS], DT, tag="a1T")
                    nc.vector.tensor_copy(a1T[:], a1T_ps[:])

                    # -- combiner layer 2: q = [q_loc | q_ls] (biased in PSUM)
                    l2 = mps_pool.tile([BS, 2 * L], F32, tag="mp")
                    for k in range(4):
                        nc.tensor.matmul(l2[:], a1T[:, k * BS:(k + 1) * BS],
                                         w["wc2"][:, k, :],
                                         start=(k == 0), stop=False)
                    nc.tensor.matmul(l2[:], w["ones_row"][:], w["bc2"][:],
                                     start=False, stop=True)

                    # -- Sqls (host lq), z = q_loc + exp(q_ls)*eps
                    nc.vector.reduce_sum(sqls[:, t:t + 1], l2[:, L:2 * L],
                                         axis=mybir.AxisListType.X)
                    ee = tmp_pool.tile([BS, L], F32, tag="ee")
                    nc.scalar.activation(ee[:], l2[:, L:2 * L], AF.Exp)
                    me = tmp_pool.tile([BS, L], F32, tag="me")
                    nc.vector.tensor_mul(me[:], ee[:], ep[:])
                    zn = tmp_pool.tile([BS, L], F32, tag="zn")
                    nc.vector.tensor_add(zn[:], me[:], l2[:, 0:L])
                    zT_ps = tps_pool.tile([128, 4 * BS], F32, tag="tp")
                    nc.tensor.transpose(zT_ps[:, :BS], zn[:], ident[:BS, :BS])
                    zT = st_pool.tile([128, BS], DT, tag="zT")
                    nc.vector.tensor_copy(zT[:], zT_ps[:, :BS])
                    nc.vector.tensor_copy(zview[:, :, t], zT[:])
                    if t == 0:
                        nc.sync.dma_start(z1t_out[:], zT[:].bitcast(F32))

                    hT_prev, cC_prev, zT_prev = hT, cC, zT

                nc.sync.dma_start(sqls_out[:], sqls[:])

            # ---------------- Phase B: batched transition/emission ----------
            with (
                tc.tile_pool(name="wB", bufs=1) as wB,
                tc.tile_pool(name="bps", bufs=3, space="PSUM") as bps_pool,
                tc.tile_pool(name="rps", bufs=1, space="PSUM") as rps_pool,
                tc.tile_pool(name="lps", bufs=2, space="PSUM") as lps_pool,
                tc.tile_pool(name="bsb", bufs=2) as bsb_pool,
                tc.tile_pool(name="bio", bufs=3) as bio_pool,
            ):
                w = {}
                for name in BATCH_W:
                    dten = din[name]
                    tl = wB.tile(list(dten.shape), dten.dtype, tag=f"w_{name}")
                    nc.sync.dma_start(tl[:], dten[:])
                    w[name] = tl

                recon_ps = rps_pool.tile([1, 512], F32, tag="recon")

                for c in range(NCH):
                    mvp = zbuf[:, c * 512:(c + 1) * 512]          # z_{t-1}
                    mvz = zbuf[:, c * 512 + 1:(c + 1) * 512 + 1]  # z_t

                    for pass_i, (w0, w1, w2, b0c, b1c, lsnc, lsc) in enumerate([
                        ("wt0", "wt1", "wt2", "bt0c", "bt1c", "btlsnc", "btlsc"),
                        ("we0", "we1", "we2", "be0c", "be1c", "belsnc", "belsc"),
                    ]):
                        mv = mvp if pass_i == 0 else mvz
                        # layer 0
                        a0b = []
                        for j in range(4):
                            p = bps_pool.tile([128, 512], F32, tag="bp")
                            nc.tensor.matmul(p[:], w[w0][:, j * 128:(j + 1) * 128],
                                             mv, start=True, stop=True)
                            a = bsb_pool.tile([128, 512], DT, tag=f"a0b{j}")
                            nc.scalar.activation(a[:], p[:], AF.Relu,
                                                 bias=w[b0c][:, j:j + 1])
                            a0b.append(a)
                        # layer 1
                        a1b = []
                        for j in range(4):
                            p = bps_pool.tile([128, 512], F32, tag="bp")
                            for k in range(4):
                                nc.tensor.matmul(
                                    p[:], w[w1][:, k, j * 128:(j + 1) * 128],
                                    a0b[k][:], start=(k == 0), stop=(k == 3))
                            a = bsb_pool.tile([128, 512], DT, tag=f"a1b{j}")
                            nc.scalar.activation(a[:], p[:], AF.Relu,
                                                 bias=w[b1c][:, j:j + 1])
                            a1b.append(a)
                        # layer 2: loc, ls
                        ploc = bps_pool.tile([128, 512], F32, tag="bp")
                        for k in range(4):
                            nc.tensor.matmul(ploc[:], w[w2][:, k, 0:128],
                                             a1b[k][:], start=(k == 0), stop=(k == 3))
                        pls = bps_pool.tile([128, 512], F32, tag="bp")
                        for k in range(4):
                            nc.tensor.matmul(pls[:], w[w2][:, k, 128:256],
                                             a1b[k][:], start=(k == 0), stop=(k == 3))

                        # s = ((tgt - loc - bloc)*exp(-ls - bls))^2 + 2(ls + bls)
                        ee = bsb_pool.tile([128, 512], F32, tag="bee")
                        nc.scalar.activation(ee[:], pls[:], AF.Exp,
                                             scale=-1.0, bias=w[lsnc][:])
                        t1 = bsb_pool.tile([128, 512], F32, tag="bt1")
                        rr = bsb_pool.tile([128, 512], F32, tag="brr")
                        if pass_i == 0:
                            nc.vector.tensor_sub(t1[:], mvz, ploc[:])
                            nc.vector.scalar_tensor_tensor(
                                rr[:], t1[:], w["btlocc"][:], ee[:],
                                op0=ALU.subtract, op1=ALU.mult)
                        else:
                            xc = bio_pool.tile([128, 512], F32, tag="xc")
                            nc.sync.dma_start(xc[:], xadj[:, c * 512:(c + 1) * 512])
                            nc.vector.tensor_sub(t1[:], xc[:], ploc[:])
                            nc.vector.tensor_mul(rr[:], t1[:], ee[:])
                        sq = bsb_pool.tile([128, 512], F32, tag="bsq")
                        nc.vector.tensor_mul(sq[:], rr[:], rr[:])
                        s1 = bsb_pool.tile([128, 512], F32, tag="bs1")
                        nc.vector.tensor_scalar(
                            s1[:], pls[:], w[lsc][:], 2.0,
                            op0=ALU.add, op1=ALU.mult)
                        ss = bsb_pool.tile([128, 512], DT, tag="bss")
                        nc.vector.tensor_add(ss[:], sq[:], s1[:])

                        if pass_i == 0:
                            lpp = lps_pool.tile([1, 512], F32, tag="lpp")
                            nc.tensor.matmul(lpp[:], w["ones_col"][:], ss[:],
                                             start=True, stop=True)
                            lsb = bio_pool.tile([1, 512], F32, tag="lsb")
                            nc.scalar.copy(lsb[:], lpp[:])
                            nc.sync.dma_start(lp_out[:, c * 512:(c + 1) * 512], lsb[:])
                        else:
                            nc.tensor.matmul(recon_ps[:], w["ones_col"][:], ss[:],
                                             start=(c == 0), stop=(c == NCH - 1))

                rfin = bio_pool.tile([1, 512], F32, tag="rfin")
                nc.scalar.copy(rfin[:], recon_ps[:])
                nc.sync.dma_start(recon_out[:], rfin[:])

    nc.compile()
    return nc


def _prep_shared(inputs):
    """Host-side weight preprocessing (shared across cores)."""
    f = {k: np.asarray(v, np.float32) for k, v in inputs.items()}
    # gate reorder [i, f, g, o] -> [i, f, o, g]
    perm = np.concatenate([np.arange(0, H), np.arange(H, 2 * H),
                           np.arange(3 * H, 4 * H), np.arange(2 * H, 3 * H)])

    def ktiles(a):  # (K, N) -> (128, K//128, N)
        K, N = a.shape
        return np.ascontiguousarray(a.reshape(K // 128, 128, N).transpose(1, 0, 2))

    sh = {}
    sh["wih"] = np.ascontiguousarray(f["Wih"][perm].T).astype(NPDT)
    sh["whh"] = ktiles(f["Whh"][perm].T * 0.5).astype(NPDT)
    sh["blstm"] = f["b_lstm"][perm][None, :].astype(NPDT)
    sh["wc0z"] = np.ascontiguousarray(f["Wc0"][:, :L].T).astype(NPDT)
    sh["wc0h"] = ktiles(f["Wc0"][:, L:].T * 0.5).astype(NPDT)
    sh["bc0"] = f["bc0"][None, :].astype(NPDT)
    sh["wc1"] = ktiles(f["Wc1"].T).astype(NPDT)
    sh["bc1"] = f["bc1"][None, :].astype(NPDT)
    sh["wc2"] = ktiles(f["Wc2"].T).astype(NPDT)
    sh["bc2"] = f["bc2"][None, :].astype(NPDT)
    sh["wt0"] = np.ascontiguousarray(f["Wt0"].T).astype(NPDT)
    sh["wt1"] = ktiles(f["Wt1"].T).astype(NPDT)
    sh["wt2"] = ktiles(f["Wt2"].T).astype(NPDT)
    sh["we0"] = np.ascontiguousarray(f["We0"].T).astype(NPDT)
    sh["we1"] = ktiles(f["We1"].T).astype(NPDT)
    sh["we2"] = ktiles(f["We2"].T).astype(NPDT)
    sh["ones_row"] = np.ones((1, BS), NPDT)
    sh["ones_col"] = np.ones((128, 1), NPDT)
    sh["ident"] = np.eye(128, dtype=np.float32)
    sh["bt0c"] = np.ascontiguousarray(f["bt0"].reshape(4, 128).T)
    sh["bt1c"] = np.ascontiguousarray(f["bt1"].reshape(4, 128).T)
    sh["btlocc"] = f["bt2"][:L][:, None].copy()
    sh["btlsnc"] = -f["bt2"][L:][:, None].copy()
    sh["btlsc"] = f["bt2"][L:][:, None].copy()
    sh["be0c"] = np.ascontiguousarray(f["be0"].reshape(4, 128).T)
    sh["be1c"] = np.ascontiguousarray(f["be1"].reshape(4, 128).T)
    sh["belsnc"] = -f["be2"][D:][:, None].copy()
    sh["belsc"] = f["be2"][D:][:, None].copy()
    return sh


def _run_device(nc, inputs, T, trace=False):
    x_seq = np.asarray(inputs["x_seq"], np.float32)
    eps = np.asarray(inputs["eps"], np.float32)
    sh = _prep_shared(inputs)
    be2loc = np.asarray(inputs["be2"], np.float32)[:D]

    in_maps = []
    for ci in range(NCORES):
        bsl = slice(ci * BS, (ci + 1) * BS)
        xs = x_seq[bsl]  # (BS, T, D)
        m = dict(sh)
        m["xT"] = np.ascontiguousarray(xs.transpose(2, 1, 0)).astype(NPDT)
        m["xadj"] = np.ascontiguousarray(
            (xs.transpose(2, 0, 1) - be2loc[:, None, None]).reshape(D, BS * T))
        m["epsn"] = np.ascontiguousarray(eps[:, bsl, :])
        in_maps.append(m)

    return run_bass_kernel_spmd(nc, in_maps, list(range(NCORES)), trace=trace)


def _finish_host(res, inputs, T):
    eps = np.asarray(inputs["eps"], np.float32)
    z1_loc = np.asarray(inputs["z1_loc"], np.float64)
    z1_ls = np.asarray(inputs["z1_logscale"], np.float64)
    total = 0.0
    for ci in range(NCORES):
        r = res.results[ci]
        bsl = slice(ci * BS, (ci + 1) * BS)
        sqls = r["sqls_out"].astype(np.float64)            # (BS, T)
        lp_raw = r["lp_out"].astype(np.float64).reshape(BS, T)
        z1 = r["z1t_out"].astype(np.float64).T             # (BS, 128)

        eps2 = (eps[:, bsl, :].astype(np.float64) ** 2).sum(-1).T  # (BS, T)
        lq = -0.5 * (eps2 + 2.0 * sqls + L * LOG2PI)
        lp = np.empty_like(lq)
        lp[:, 1:] = -0.5 * (lp_raw[:, 1:] + L * LOG2PI)
        lp[:, 0] = -0.5 * (
            ((z1 - z1_loc) * np.exp(-z1_ls)) ** 2 + 2.0 * z1_ls + LOG2PI
        ).sum(-1)
        kl = np.maximum(lq - lp, FREE_BITS).sum()
        recon = -0.5 * (r["recon_out"].astype(np.float64).sum() + BS * T * D * LOG2PI)
        total += recon - kl

    return np.float32(-total / B)


def kernel(**inputs):
    x_seq = np.asarray(inputs["x_seq"], np.float32)
    T = x_seq.shape[1]
    if T not in _CACHE:
        _CACHE[T] = _build(T)
    nc = _CACHE[T]
    res = _run_device(nc, inputs, T)
    return _finish_host(res, inputs, T)
